# revision 32
# baseline (speedup 1.0000x reference)
"""Multi-head causal self-attention on 8 Trainium2 NeuronCores.

Problem: B=4, T=2048, D=1024, H=16 heads, Hd=64. fp32.
Sharding: core c handles batch b = c//2 and head-group g = c%2 (8 heads,
512 channels). Each core computes a partial output (its head-group's
contribution to x @ Wo); the host sums head-group pairs and adds bo.

Per-core algorithm (all layouts chosen so no on-chip transposes are
needed after the single x -> x^T transpose):
  x^T  [D=1024, T]   via XBAR DMA transpose (weights are DMA'd first so
                     nothing queues behind the 16 transposes)
  Q^T  [C=512, T]    = matmul(lhsT=Wq chunk, rhs=x^T)   (head h at partitions
  K^T  [C=512, T]      64*(h%2) .. of chunk h//2)
  V'   [T, 8*65]     = matmul(lhsT=x^T chunk, rhs=Wv), per head [V(64) | 1]
  S^T  [k,q]         = matmul(lhsT=K^T block, rhs=Q^T span); the two heads
                       of a pair are row-tiled (partitions 0-63 / 64-127)
                       and run concurrently in the PE array
  E = exp(S^T/8)     on ScalarE, PSUM->SBUF; diagonal blocks get a
                     multiplicative staircase mask on their first 128
                     columns only (the rest of the window is always valid)
  ctx' [65, q]       = matmul(lhsT=V' block, rhs=E)  accumulated over k blocks
                       row 64 = softmax denominator (ones-column trick)
  ctx^T normalized via reciprocal + DRAM-bounce partition broadcast; the
                     ctx' PSUM accumulators are copied to SBUF first so the
                     single PSUM bank per head frees immediately
  out  [T, D]        = matmul(lhsT=ctx^T chunk, rhs=Wo chunk), bf16 DMA out

Phase C (attention) is ScalarE-exp-throughput-bound (~1us/k-block), so
the emission order feeds the Tile scheduler coarse filler blocks it can
slot into C's PE idle time:
  - K/Q/V' groups for (hp0, span s) are emitted just before C(hp0, s), so
    the first exp starts ~25us into the kernel;
  - QK groups for hp 1..3 are emitted between the C(hp) phases and get
    scheduled into earlier C spans' PE gaps;
  - hp3 runs its spans DESCENDING with the output projection for span s
    emitted right after norm(3, s), so out-proj overlaps the remaining
    attention work instead of serializing at the end.
PSUM budget: shared V'/QK/out-proj pool 2 banks + S^T 4 + ctx' 2 = 8.
Causality: only k-blocks with k0 <= q_span_end are computed.
"""

import sys

for _p in ("/opt/trn_rl_repo", "/root/.axon_site/_ro/trn_rl_repo"):
    if _p not in sys.path:
        sys.path.append(_p)

import numpy as np

import concourse.bacc as bacc
import concourse.mybir as mybir
import concourse.tile as tile
from concourse.bass_utils import run_bass_kernel_spmd

FP32 = mybir.dt.float32
BF16 = mybir.dt.bfloat16
P = 128
T = 2048  # sequence length
D = 1024  # model dim
C = 512   # channels per core (8 heads)
H = 8     # heads per core
HD = 64   # head dim
N_CORES = 8
NSPAN = 4          # q spans of 512
SPAN = 512
NKB = 16           # k blocks of 128

_program = None


def _build():
    nc = bacc.Bacc()
    # x arrives pre-transposed AND pre-tiled from the host as
    # [span, 128 d-part, 8 d-chunk, 512 t] so each span's x^T is one
    # fully contiguous 1MB DMA (no XBAR transpose, no strided descriptors).
    x_d = nc.declare_dram_parameter("x", [NSPAN, P, 8, SPAN], BF16,
                                    isOutput=False)
    wq_d = nc.declare_dram_parameter("wq", [D, C], BF16, isOutput=False)
    wk_d = nc.declare_dram_parameter("wk", [D, C], BF16, isOutput=False)
    wv_d = nc.declare_dram_parameter("wv", [D, C], BF16, isOutput=False)
    wo_d = nc.declare_dram_parameter("wo", [C, D], BF16, isOutput=False)
    mask_d = nc.declare_dram_parameter("mask", [P, P], BF16, isOutput=False)
    out_d = nc.declare_dram_parameter("out", [T, D], BF16, isOutput=True)

    Exp = mybir.ActivationFunctionType.Exp

    from contextlib import ExitStack

    with tile.TileContext(nc) as tc, ExitStack() as persist:
        const_pool = persist.enter_context(tc.tile_pool(name="const", bufs=1))
        qkt_pool = persist.enter_context(tc.tile_pool(name="qkt", bufs=1))
        vp_pool = persist.enter_context(tc.tile_pool(name="vp", bufs=1))
        persist_w = persist.enter_context(tc.tile_pool(name="pw", bufs=1))
        ctxT_pool = persist.enter_context(tc.tile_pool(name="ctxT", bufs=1))
        xt_pool = persist.enter_context(tc.tile_pool(name="xt", bufs=1))

        # ---- persistent SBUF tiles --------------------------------------
        mask_sb = const_pool.tile([P, P], BF16, tag="mask")
        wv_sb = persist_w.tile([P, 8, C], BF16, tag="wv")
        wq_sb = persist_w.tile([P, 8, C], BF16, tag="wq")
        wk_sb = persist_w.tile([P, 8, C], BF16, tag="wk")
        wo_sb = persist_w.tile([P, 4, D], BF16, tag="wo")
        qt = [qkt_pool.tile([P, T], BF16, tag=f"qt{i}", name=f"qt{i}") for i in range(4)]
        kt = [qkt_pool.tile([P, T], BF16, tag=f"kt{i}", name=f"kt{i}") for i in range(4)]
        vp = [vp_pool.tile([P, H * 65], BF16, tag=f"vp{t}", name=f"vp{t}") for t in range(NKB)]
        ctxT = [ctxT_pool.tile([P, T], BF16, tag=f"ct{i}", name=f"ct{i}")
                for i in range(4)]
        # x^T as one tile per q-span: [128 d-part, 8 d-chunk, 512 t].
        # One 1MB DMA per span matches the per-span consumption pattern
        # (DMA completion lags issue by several us and the 8 semaphore
        # lanes serialize on reuse, so fewer/bigger DMAs win at startup).
        xsp = [xt_pool.tile([P, 8, SPAN], BF16, tag=f"xs{s}", name=f"xs{s}")
               for s in range(NSPAN)]
        warm_sb = const_pool.tile([P, SPAN], BF16, tag="warm")

        # ---- startup DMAs: weights on the sync ring, x spans on the
        # scalar ring so the two 1MB streams transfer in parallel
        # (consecutive DMAs on one ring serialize at the data level).
        nc.sync.dma_start(wk_sb[:], wk_d.rearrange("(o p) c -> p o c", p=P))
        nc.sync.dma_start(wq_sb[:], wq_d.rearrange("(o p) c -> p o c", p=P))
        nc.sync.dma_start(mask_sb[:], mask_d[:])
        nc.sync.dma_start(wv_sb[:], wv_d.rearrange("(o p) c -> p o c", p=P))
        nc.sync.dma_start(wo_sb[:], wo_d.rearrange("(o p) d -> p o d", p=P))
        for s in range(NSPAN):
            nc.scalar.dma_start(xsp[s][:], x_d[s])

        # ones columns of V' (value 1.0 at element 64 of each head block);
        # warm_sb feeds the PE warm-up burst below.
        nc.gpsimd.memset(warm_sb[:], 1.0)
        for t in range(NKB):
            nc.gpsimd.memset(vp[t][:], 1.0)

        with (
            tc.tile_pool(name="proj", bufs=2, space="PSUM") as proj_pool,
            tc.tile_pool(name="stps", bufs=2, space="PSUM") as st_pool,
            tc.tile_pool(name="csA", bufs=1, space="PSUM") as csA_pool,
            tc.tile_pool(name="csB", bufs=1, space="PSUM") as csB_pool,
            tc.tile_pool(name="epool", bufs=6) as e_pool,
            tc.tile_pool(name="npool", bufs=2) as n_pool,
            tc.tile_pool(name="rdram", bufs=2, space="DRAM") as rdram_pool,
            tc.tile_pool(name="opool", bufs=2) as o_pool,
        ):
            def emit_warmup(n):
                # dummy matmuls to keep the PE HAM clock gate at 2.4 GHz
                # across stretches where the PE would otherwise idle
                ps = proj_pool.tile([P, SPAN], FP32, tag="proj")
                for r in range(n):
                    nc.tensor.matmul(ps[:], warm_sb[:, 0:P], warm_sb[:],
                                     start=(r == 0), stop=(r == n - 1))

            def emit_vprime(t):
                # V' for token block t: [128t, 8*65] with ones col at 64
                sp, tc = t // 4, t % 4
                ps = proj_pool.tile([P, C], FP32, tag="proj")
                for j in range(8):
                    nc.tensor.matmul(
                        ps[:],
                        xsp[sp][:, j, tc * P:(tc + 1) * P],
                        wv_sb[:, j, :],
                        start=(j == 0), stop=(j == 7),
                    )
                dst = vp[t].rearrange("p (h e) -> p h e", e=65)[:, :, 0:64]
                nc.vector.tensor_copy(dst, ps.rearrange("p (h e) -> p h e", e=64))

            def emit_qk_group(dst, wsb, hp, s):
                ps = proj_pool.tile([P, SPAN], FP32, tag="proj")
                for j in range(8):
                    nc.tensor.matmul(
                        ps[:],
                        wsb[:, j, hp * P:(hp + 1) * P],
                        xsp[s][:, j, :],
                        start=(j == 0), stop=(j == 7),
                    )
                nc.vector.tensor_copy(dst[hp][:, s * SPAN:(s + 1) * SPAN], ps[:])

            def emit_attn_span(hp, s, last=False):
                hA, hB = 2 * hp, 2 * hp + 1
                csA = csA_pool.tile([P, SPAN], FP32, tag="csA")
                csB = csB_pool.tile([P, SPAN], FP32, tag="csB")
                nkb = 4 * s + 4
                for kb in range(nkb):
                    ksl = slice(kb * P, (kb + 1) * P)
                    d = max(0, kb - 4 * s)      # diagonal offset 0..3
                    q0 = s * SPAN + 128 * d     # valid q start
                    w = SPAN - 128 * d          # valid width
                    qsl = slice(q0, (s + 1) * SPAN)
                    st = st_pool.tile([P, 1024], FP32, tag="st")
                    st3 = st.rearrange("p (b q) -> p b q", b=2)[:, :, 0:w]
                    # the two heads run concurrently (row-tiled at
                    # partitions 0-63 / 64-127)
                    nc.tensor.matmul(st[:, 0:w], kt[hp][0:64, ksl],
                                     qt[hp][0:64, qsl],
                                     start=True, stop=True)
                    nc.tensor.matmul(st[:, 512:512 + w], kt[hp][64:128, ksl],
                                     qt[hp][64:128, qsl],
                                     start=True, stop=True)
                    e = e_pool.tile([P, 1024], BF16, tag="e")
                    e3 = e.rearrange("p (b q) -> p b q", b=2)[:, :, 0:w]
                    nc.scalar.activation(e3, st3, Exp, scale=0.125)
                    if d > 0 or kb == 4 * s:
                        # staircase only affects the first 128 columns of
                        # the valid window (beyond that q-k >= 128 always)
                        e3m = e.rearrange("p (b q) -> p b q", b=2)[:, :, 0:128]
                        m3 = mask_sb[:, None, :]
                        nc.vector.tensor_mul(
                            e3m, e3m, m3.to_broadcast((P, 2, 128)))
                    co = 128 * d
                    nc.tensor.matmul(csA[0:65, co:SPAN],
                                     vp[kb][:, hA * 65:(hA + 1) * 65],
                                     e[:, 0:w],
                                     start=(kb == 0), stop=(kb == nkb - 1))
                    nc.tensor.matmul(csB[0:65, co:SPAN],
                                     vp[kb][:, hB * 65:(hB + 1) * 65],
                                     e[:, 512:512 + w],
                                     start=(kb == 0), stop=(kb == nkb - 1))
                # Copy the accumulators to SBUF immediately so the csA/csB
                # banks free for the next span; normalize from the copy.
                # rows 0..63 / row 64 (ones-column rowsum).
                # reciprocal_approx_fast is broken at nonzero base
                # partition: broadcast first (DRAM bounce), recip at 0.
                qsl = slice(s * SPAN, (s + 1) * SPAN)
                cs = n_pool.tile([P, 1024], FP32, tag="cs")
                rsAB = n_pool.tile([P, 1024], FP32, tag="rsAB")
                rrAB = n_pool.tile([P, 1024], FP32, tag="rrAB")
                tmpB = n_pool.tile([P, SPAN], BF16, tag="tmpB")
                nc.vector.tensor_copy(cs[0:65, 0:512], csA[0:65, :])
                nc.vector.tensor_copy(cs[0:65, 512:1024], csB[0:65, :])
                rd = rdram_pool.tile([1024], FP32, tag="rd")
                # on the final span ScalarE is done -- use its idle HWDGE
                # ring so the tail chain avoids sync-ring queueing
                dma_eng = nc.scalar if last else nc.sync
                dma_eng.dma_start(rd[None, :], cs[64:65, :])
                dma_eng.dma_start(
                    rsAB[0:64, :], rd[None, :].to_broadcast((64, 1024)))
                nc.vector.reciprocal_approx_fast(rrAB[0:64, :], rsAB[0:64, :])
                nc.vector.tensor_mul(ctxT[hp][0:64, qsl],
                                     cs[0:64, 0:512], rrAB[0:64, 0:512])
                nc.vector.tensor_mul(tmpB[0:64, :],
                                     cs[0:64, 512:1024], rrAB[0:64, 512:1024])
                dma_eng.dma_start(ctxT[hp][64:128, qsl], tmpB[0:64, :])

            def emit_out_span(s):
                # output projection for the 4 token blocks of span s
                for qb in range(4 * s, 4 * s + 4):
                    ot = o_pool.tile([P, 2, SPAN], BF16, tag="ot")
                    for nh in range(2):
                        ps = proj_pool.tile([P, SPAN], FP32, tag="proj")
                        for hp in range(4):
                            nc.tensor.matmul(
                                ps[:],
                                ctxT[hp][:, qb * P:(qb + 1) * P],
                                wo_sb[:, hp, nh * SPAN:(nh + 1) * SPAN],
                                start=(hp == 0), stop=(hp == 3),
                            )
                        nc.vector.tensor_copy(ot[:, nh, :], ps[:])
                    eng = nc.scalar if s == NSPAN - 1 and qb % 2 == 1 else nc.sync
                    eng.dma_start(
                        out_d[qb * P:(qb + 1) * P, :],
                        ot.rearrange("p a b -> p (a b)"))

            # ---- emission schedule ------------------------------------
            # hp-major (emission order IS per-engine execution order, so
            # keep per-boundary lead-ins small): each head pair's K/Q
            # groups are emitted per span just before that span's
            # attention; hp3 runs its spans DESCENDING with the output
            # projection for span s right after norm(3, s) so out-proj
            # overlaps the remaining attention instead of the tail.
            emit_warmup(28)
            for s in range(NSPAN):
                emit_qk_group(kt, wk_sb, 0, s)
                emit_qk_group(qt, wq_sb, 0, s)
                for t in range(4 * s, 4 * s + 4):
                    emit_vprime(t)
                emit_attn_span(0, s)
            for hp in (1, 2):
                for s in range(NSPAN):
                    emit_qk_group(kt, wk_sb, hp, s)
                    emit_qk_group(qt, wq_sb, hp, s)
                    emit_attn_span(hp, s)
            for s in range(NSPAN):
                emit_qk_group(kt, wk_sb, 3, s)
                emit_qk_group(qt, wq_sb, 3, s)
            for s in reversed(range(NSPAN)):
                emit_attn_span(3, s, last=(s == 0))
                if s == 0:
                    # bridge the final norm-chain latency so the PE stays
                    # warm for the last output-projection burst
                    emit_warmup(16)
                emit_out_span(s)

    nc.compile()
    return nc


def _get_program():
    global _program
    if _program is None:
        _program = _build()
    return _program


def _make_mask():
    import ml_dtypes
    j = np.arange(P)[None, :]
    k = np.arange(P)[:, None]
    return np.where(j >= k, 1.0, 0.0).astype(ml_dtypes.bfloat16)


def _make_in_maps(x, Wq, Wk, Wv, Wo):
    import ml_dtypes
    bf16 = ml_dtypes.bfloat16
    mask = _make_mask()
    in_maps = []
    xp = {}
    for b in range(x.shape[0]):
        # [T, D] -> x^T tiled as [span, 128 d-part, 8 d-chunk, 512 t]
        xT = np.asarray(x[b], np.float32).astype(bf16).T  # [D, T]
        xp[b] = np.ascontiguousarray(
            xT.reshape(8, P, NSPAN, SPAN).transpose(2, 1, 0, 3))
    for c in range(N_CORES):
        b, g = c // 2, c % 2
        cols = slice(g * C, (g + 1) * C)
        in_maps.append({
            "x": xp[b],
            "wq": np.ascontiguousarray(np.asarray(Wq[:, cols], np.float32).astype(bf16)),
            "wk": np.ascontiguousarray(np.asarray(Wk[:, cols], np.float32).astype(bf16)),
            "wv": np.ascontiguousarray(np.asarray(Wv[:, cols], np.float32).astype(bf16)),
            "wo": np.ascontiguousarray(np.asarray(Wo[cols, :], np.float32).astype(bf16)),
            "mask": mask,
        })
    return in_maps


def _combine(results, bo, B):
    out = np.empty((B, T, D), dtype=np.float32)
    bo = np.asarray(bo, dtype=np.float32)
    for b in range(B):
        out[b] = (results[2 * b]["out"].astype(np.float32)
                  + results[2 * b + 1]["out"].astype(np.float32) + bo)
    return out


def kernel(x, Wq, Wk, Wv, Wo, bo):
    x = np.asarray(x)
    nc = _get_program()
    in_maps = _make_in_maps(x, Wq, Wk, Wv, Wo)
    res = run_bass_kernel_spmd(nc, in_maps, core_ids=list(range(N_CORES)))
    return _combine(res.results, bo, x.shape[0])


def kernel_traced(x, Wq, Wk, Wv, Wo, bo):
    """Like kernel() but also returns the BassKernelResults (with
    exec_time_ns when NTFF tracing is available)."""
    x = np.asarray(x)
    nc = _get_program()
    in_maps = _make_in_maps(x, Wq, Wk, Wv, Wo)
    res = run_bass_kernel_spmd(nc, in_maps, core_ids=list(range(N_CORES)),
                               trace=True)
    return _combine(res.results, bo, x.shape[0]), res


# revision 37
# speedup vs baseline: 1.0148x; 1.0148x over previous
"""Multi-head causal self-attention on 8 Trainium2 NeuronCores.

Problem: B=4, T=2048, D=1024, H=16 heads, Hd=64. fp32.
Sharding: core c handles batch b = c//2 and head-group g = c%2 (8 heads,
512 channels). Each core computes a partial output (its head-group's
contribution to x @ Wo); the host sums head-group pairs and adds bo.

Per-core algorithm (all layouts chosen so no on-chip transposes are
needed after the single x -> x^T transpose):
  x^T  [D=1024, T]   via XBAR DMA transpose (weights are DMA'd first so
                     nothing queues behind the 16 transposes)
  Q^T  [C=512, T]    = matmul(lhsT=Wq chunk, rhs=x^T)   (head h at partitions
  K^T  [C=512, T]      64*(h%2) .. of chunk h//2)
  V'   [T, 8*65]     = matmul(lhsT=x^T chunk, rhs=Wv), per head [V(64) | 1]
  S^T  [k,q]         = matmul(lhsT=K^T block, rhs=Q^T span); the two heads
                       of a pair are row-tiled (partitions 0-63 / 64-127)
                       and run concurrently in the PE array
  E = exp(S^T/8)     on ScalarE, PSUM->SBUF; diagonal blocks get a
                     multiplicative staircase mask on their first 128
                     columns only (the rest of the window is always valid)
  ctx' [65, q]       = matmul(lhsT=V' block, rhs=E)  accumulated over k blocks
                       row 64 = softmax denominator (ones-column trick)
  ctx^T normalized via reciprocal + DRAM-bounce partition broadcast; the
                     ctx' PSUM accumulators are copied to SBUF first so the
                     single PSUM bank per head frees immediately
  out  [T, D]        = matmul(lhsT=ctx^T chunk, rhs=Wo chunk), bf16 DMA out

Phase C (attention) is ScalarE-exp-throughput-bound (~1us/k-block), so
the emission order feeds the Tile scheduler coarse filler blocks it can
slot into C's PE idle time:
  - K/Q/V' groups for (hp0, span s) are emitted just before C(hp0, s), so
    the first exp starts ~25us into the kernel;
  - QK groups for hp 1..3 are emitted between the C(hp) phases and get
    scheduled into earlier C spans' PE gaps;
  - hp3 runs its spans DESCENDING with the output projection for span s
    emitted right after norm(3, s), so out-proj overlaps the remaining
    attention work instead of serializing at the end.
PSUM budget: shared V'/QK/out-proj pool 2 banks + S^T 4 + ctx' 2 = 8.
Causality: only k-blocks with k0 <= q_span_end are computed.
"""

import sys

for _p in ("/opt/trn_rl_repo", "/root/.axon_site/_ro/trn_rl_repo"):
    if _p not in sys.path:
        sys.path.append(_p)

import numpy as np

import concourse.bacc as bacc
import concourse.mybir as mybir
import concourse.tile as tile
from concourse.bass_utils import run_bass_kernel_spmd

FP32 = mybir.dt.float32
BF16 = mybir.dt.bfloat16
P = 128
T = 2048  # sequence length
D = 1024  # model dim
C = 512   # channels per core (8 heads)
H = 8     # heads per core
HD = 64   # head dim
N_CORES = 8
NSPAN = 4          # q spans of 512
SPAN = 512
NKB = 16           # k blocks of 128

_program = None


def _build():
    nc = bacc.Bacc()
    # x arrives pre-transposed AND pre-tiled from the host as
    # [span, 128 d-part, 8 d-chunk, 512 t] so each span's x^T is one
    # fully contiguous 1MB DMA (no XBAR transpose, no strided descriptors).
    x_d = nc.declare_dram_parameter("x", [NSPAN, P, 8, SPAN], BF16,
                                    isOutput=False)
    wq_d = nc.declare_dram_parameter("wq", [D, C], BF16, isOutput=False)
    wk_d = nc.declare_dram_parameter("wk", [D, C], BF16, isOutput=False)
    wv_d = nc.declare_dram_parameter("wv", [D, C], BF16, isOutput=False)
    wo_d = nc.declare_dram_parameter("wo", [C, D], BF16, isOutput=False)
    # mask is [128, 1024] (only cols 0:128 used) so its DMA moves 2KB per
    # partition -- a [128, 128] layout is descriptor-dominated (~13us)
    mask_d = nc.declare_dram_parameter("mask", [P, 1024], BF16, isOutput=False)
    out_d = nc.declare_dram_parameter("out", [T, D], BF16, isOutput=True)

    Exp = mybir.ActivationFunctionType.Exp

    from contextlib import ExitStack

    with tile.TileContext(nc) as tc, ExitStack() as persist:
        const_pool = persist.enter_context(tc.tile_pool(name="const", bufs=1))
        qkt_pool = persist.enter_context(tc.tile_pool(name="qkt", bufs=1))
        vp_pool = persist.enter_context(tc.tile_pool(name="vp", bufs=1))
        persist_w = persist.enter_context(tc.tile_pool(name="pw", bufs=1))
        ctxT_pool = persist.enter_context(tc.tile_pool(name="ctxT", bufs=1))
        xt_pool = persist.enter_context(tc.tile_pool(name="xt", bufs=1))

        # ---- persistent SBUF tiles --------------------------------------
        mask_sb = const_pool.tile([P, 1024], BF16, tag="mask")
        wv_sb = persist_w.tile([P, 8, C], BF16, tag="wv")
        wq_sb = persist_w.tile([P, 8, C], BF16, tag="wq")
        wk_sb = persist_w.tile([P, 8, C], BF16, tag="wk")
        wo_sb = persist_w.tile([P, 4, D], BF16, tag="wo")
        qt = [qkt_pool.tile([P, T], BF16, tag=f"qt{i}", name=f"qt{i}") for i in range(4)]
        kt = [qkt_pool.tile([P, T], BF16, tag=f"kt{i}", name=f"kt{i}") for i in range(4)]
        vp = [vp_pool.tile([P, H * 65], BF16, tag=f"vp{t}", name=f"vp{t}") for t in range(NKB)]
        ctxT = [ctxT_pool.tile([P, T], BF16, tag=f"ct{i}", name=f"ct{i}")
                for i in range(4)]
        # x^T as one tile per q-span: [128 d-part, 8 d-chunk, 512 t].
        # One 1MB DMA per span matches the per-span consumption pattern
        # (DMA completion lags issue by several us and the 8 semaphore
        # lanes serialize on reuse, so fewer/bigger DMAs win at startup).
        xsp = [xt_pool.tile([P, 8, SPAN], BF16, tag=f"xs{s}", name=f"xs{s}")
               for s in range(NSPAN)]
        warm_sb = const_pool.tile([P, SPAN], BF16, tag="warm")

        # ---- startup DMAs: one ring, strict consumption order (startup
        # is HBM-bandwidth-bound at ~8MB total, so what matters is that
        # the first-needed 3MB -- wk, wq, x span 0 -- transfer first).
        nc.sync.dma_start(wk_sb[:], wk_d.rearrange("(o p) c -> p o c", p=P))
        nc.sync.dma_start(wq_sb[:], wq_d.rearrange("(o p) c -> p o c", p=P))
        nc.sync.dma_start(xsp[0][:], x_d[0])
        nc.sync.dma_start(wv_sb[:], wv_d.rearrange("(o p) c -> p o c", p=P))
        nc.sync.dma_start(mask_sb[:], mask_d[:])
        for s in range(1, NSPAN):
            nc.sync.dma_start(xsp[s][:], x_d[s])
        nc.sync.dma_start(wo_sb[:], wo_d.rearrange("(o p) d -> p o d", p=P))

        # ones columns of V' (value 1.0 at element 64 of each head block);
        # warm_sb feeds the PE warm-up burst below.
        nc.gpsimd.memset(warm_sb[:], 1.0)
        for t in range(NKB):
            nc.gpsimd.memset(vp[t][:], 1.0)

        with (
            tc.tile_pool(name="proj", bufs=2, space="PSUM") as proj_pool,
            tc.tile_pool(name="stps", bufs=2, space="PSUM") as st_pool,
            tc.tile_pool(name="csA", bufs=1, space="PSUM") as csA_pool,
            tc.tile_pool(name="csB", bufs=1, space="PSUM") as csB_pool,
            tc.tile_pool(name="epool", bufs=8) as e_pool,
            tc.tile_pool(name="npool", bufs=2) as n_pool,
            tc.tile_pool(name="rdram", bufs=2, space="DRAM") as rdram_pool,
            tc.tile_pool(name="opool", bufs=2) as o_pool,
        ):
            def emit_warmup(n):
                # dummy matmuls to keep the PE HAM clock gate at 2.4 GHz
                # across stretches where the PE would otherwise idle
                ps = proj_pool.tile([P, SPAN], FP32, tag="proj")
                for r in range(n):
                    nc.tensor.matmul(ps[:], warm_sb[:, 0:P], warm_sb[:],
                                     start=(r == 0), stop=(r == n - 1))

            def emit_vprime(t):
                # V' for token block t: [128t, 8*65] with ones col at 64
                sp, tc = t // 4, t % 4
                ps = proj_pool.tile([P, C], FP32, tag="proj")
                for j in range(8):
                    nc.tensor.matmul(
                        ps[:],
                        xsp[sp][:, j, tc * P:(tc + 1) * P],
                        wv_sb[:, j, :],
                        start=(j == 0), stop=(j == 7),
                    )
                dst = vp[t].rearrange("p (h e) -> p h e", e=65)[:, :, 0:64]
                nc.vector.tensor_copy(dst, ps.rearrange("p (h e) -> p h e", e=64))

            def emit_qk_group(dst, wsb, hp, s):
                ps = proj_pool.tile([P, SPAN], FP32, tag="proj")
                for j in range(8):
                    nc.tensor.matmul(
                        ps[:],
                        wsb[:, j, hp * P:(hp + 1) * P],
                        xsp[s][:, j, :],
                        start=(j == 0), stop=(j == 7),
                    )
                nc.vector.tensor_copy(dst[hp][:, s * SPAN:(s + 1) * SPAN], ps[:])

            def emit_attn_span(hp, s, last=False):
                hA, hB = 2 * hp, 2 * hp + 1
                csA = csA_pool.tile([P, SPAN], FP32, tag="csA")
                csB = csB_pool.tile([P, SPAN], FP32, tag="csB")
                nkb = 4 * s + 4
                for kb in range(nkb):
                    ksl = slice(kb * P, (kb + 1) * P)
                    d = max(0, kb - 4 * s)      # diagonal offset 0..3
                    q0 = s * SPAN + 128 * d     # valid q start
                    w = SPAN - 128 * d          # valid width
                    qsl = slice(q0, (s + 1) * SPAN)
                    st = st_pool.tile([P, 1024], FP32, tag="st")
                    st3 = st.rearrange("p (b q) -> p b q", b=2)[:, :, 0:w]
                    # the two heads run concurrently (row-tiled at
                    # partitions 0-63 / 64-127)
                    nc.tensor.matmul(st[:, 0:w], kt[hp][0:64, ksl],
                                     qt[hp][0:64, qsl],
                                     start=True, stop=True)
                    nc.tensor.matmul(st[:, 512:512 + w], kt[hp][64:128, ksl],
                                     qt[hp][64:128, qsl],
                                     start=True, stop=True)
                    e = e_pool.tile([P, 1024], BF16, tag="e")
                    e3 = e.rearrange("p (b q) -> p b q", b=2)[:, :, 0:w]
                    nc.scalar.activation(e3, st3, Exp, scale=0.125)
                    if d > 0 or kb == 4 * s:
                        # staircase only affects the first 128 columns of
                        # the valid window (beyond that q-k >= 128 always)
                        e3m = e.rearrange("p (b q) -> p b q", b=2)[:, :, 0:128]
                        m3 = mask_sb[:, None, 0:128]
                        nc.vector.tensor_mul(
                            e3m, e3m, m3.to_broadcast((P, 2, 128)))
                    co = 128 * d
                    nc.tensor.matmul(csA[0:65, co:SPAN],
                                     vp[kb][:, hA * 65:(hA + 1) * 65],
                                     e[:, 0:w],
                                     start=(kb == 0), stop=(kb == nkb - 1))
                    nc.tensor.matmul(csB[0:65, co:SPAN],
                                     vp[kb][:, hB * 65:(hB + 1) * 65],
                                     e[:, 512:512 + w],
                                     start=(kb == 0), stop=(kb == nkb - 1))
                # Copy the accumulators to SBUF immediately so the csA/csB
                # banks free for the next span; normalize from the copy.
                # rows 0..63 / row 64 (ones-column rowsum).
                # reciprocal_approx_fast is broken at nonzero base
                # partition: broadcast first (DRAM bounce), recip at 0.
                qsl = slice(s * SPAN, (s + 1) * SPAN)
                cs = n_pool.tile([P, 1024], FP32, tag="cs")
                rsAB = n_pool.tile([P, 1024], FP32, tag="rsAB")
                rrAB = n_pool.tile([P, 1024], FP32, tag="rrAB")
                tmpB = n_pool.tile([P, SPAN], BF16, tag="tmpB")
                nc.vector.tensor_copy(cs[0:65, 0:512], csA[0:65, :])
                nc.vector.tensor_copy(cs[0:65, 512:1024], csB[0:65, :])
                rd = rdram_pool.tile([1024], FP32, tag="rd")
                nc.sync.dma_start(rd[None, :], cs[64:65, :])
                nc.sync.dma_start(
                    rsAB[0:64, :], rd[None, :].to_broadcast((64, 1024)))
                nc.vector.reciprocal_approx_fast(rrAB[0:64, :], rsAB[0:64, :])
                nc.vector.tensor_mul(ctxT[hp][0:64, qsl],
                                     cs[0:64, 0:512], rrAB[0:64, 0:512])
                nc.vector.tensor_mul(tmpB[0:64, :],
                                     cs[0:64, 512:1024], rrAB[0:64, 512:1024])
                nc.sync.dma_start(ctxT[hp][64:128, qsl], tmpB[0:64, :])

            def emit_out_span(s):
                # output projection for the 4 token blocks of span s
                for qb in range(4 * s, 4 * s + 4):
                    ot = o_pool.tile([P, 2, SPAN], BF16, tag="ot")
                    for nh in range(2):
                        ps = proj_pool.tile([P, SPAN], FP32, tag="proj")
                        for hp in range(4):
                            nc.tensor.matmul(
                                ps[:],
                                ctxT[hp][:, qb * P:(qb + 1) * P],
                                wo_sb[:, hp, nh * SPAN:(nh + 1) * SPAN],
                                start=(hp == 0), stop=(hp == 3),
                            )
                        nc.vector.tensor_copy(ot[:, nh, :], ps[:])
                    eng = nc.scalar if s == NSPAN - 1 and qb % 2 == 1 else nc.sync
                    eng.dma_start(
                        out_d[qb * P:(qb + 1) * P, :],
                        ot.rearrange("p a b -> p (a b)"))

            # ---- emission schedule ------------------------------------
            # hp-major (emission order IS per-engine execution order, so
            # keep per-boundary lead-ins small): each head pair's K/Q
            # groups are emitted per span just before that span's
            # attention; hp3 runs its spans DESCENDING with the output
            # projection for span s right after norm(3, s) so out-proj
            # overlaps the remaining attention instead of the tail.
            emit_warmup(28)
            for s in range(NSPAN):
                emit_qk_group(kt, wk_sb, 0, s)
                emit_qk_group(qt, wq_sb, 0, s)
                for t in range(4 * s, 4 * s + 4):
                    emit_vprime(t)
                emit_attn_span(0, s)
            for hp in (1, 2):
                for s in range(NSPAN):
                    emit_qk_group(kt, wk_sb, hp, s)
                    emit_qk_group(qt, wq_sb, hp, s)
                    emit_attn_span(hp, s)
            for s in range(NSPAN):
                emit_qk_group(kt, wk_sb, 3, s)
                emit_qk_group(qt, wq_sb, 3, s)
            for s in reversed(range(NSPAN)):
                emit_attn_span(3, s, last=(s == 0))
                if s == 0:
                    # bridge the final norm-chain latency so the PE stays
                    # warm for the last output-projection burst
                    emit_warmup(32)
                emit_out_span(s)

    nc.compile()
    return nc


def _get_program():
    global _program
    if _program is None:
        _program = _build()
    return _program


def _make_mask():
    import ml_dtypes
    j = np.arange(1024)[None, :]
    k = np.arange(P)[:, None]
    return np.where((j >= k) | (j >= P), 1.0, 0.0).astype(ml_dtypes.bfloat16)


def _make_in_maps(x, Wq, Wk, Wv, Wo):
    import ml_dtypes
    bf16 = ml_dtypes.bfloat16
    mask = _make_mask()
    in_maps = []
    xp = {}
    for b in range(x.shape[0]):
        # [T, D] -> x^T tiled as [span, 128 d-part, 8 d-chunk, 512 t]
        xT = np.asarray(x[b], np.float32).astype(bf16).T  # [D, T]
        xp[b] = np.ascontiguousarray(
            xT.reshape(8, P, NSPAN, SPAN).transpose(2, 1, 0, 3))
    for c in range(N_CORES):
        b, g = c // 2, c % 2
        cols = slice(g * C, (g + 1) * C)
        in_maps.append({
            "x": xp[b],
            "wq": np.ascontiguousarray(np.asarray(Wq[:, cols], np.float32).astype(bf16)),
            "wk": np.ascontiguousarray(np.asarray(Wk[:, cols], np.float32).astype(bf16)),
            "wv": np.ascontiguousarray(np.asarray(Wv[:, cols], np.float32).astype(bf16)),
            "wo": np.ascontiguousarray(np.asarray(Wo[cols, :], np.float32).astype(bf16)),
            "mask": mask,
        })
    return in_maps


def _combine(results, bo, B):
    out = np.empty((B, T, D), dtype=np.float32)
    bo = np.asarray(bo, dtype=np.float32)
    for b in range(B):
        out[b] = (results[2 * b]["out"].astype(np.float32)
                  + results[2 * b + 1]["out"].astype(np.float32) + bo)
    return out


def kernel(x, Wq, Wk, Wv, Wo, bo):
    x = np.asarray(x)
    nc = _get_program()
    in_maps = _make_in_maps(x, Wq, Wk, Wv, Wo)
    res = run_bass_kernel_spmd(nc, in_maps, core_ids=list(range(N_CORES)))
    return _combine(res.results, bo, x.shape[0])


def kernel_traced(x, Wq, Wk, Wv, Wo, bo):
    """Like kernel() but also returns the BassKernelResults (with
    exec_time_ns when NTFF tracing is available)."""
    x = np.asarray(x)
    nc = _get_program()
    in_maps = _make_in_maps(x, Wq, Wk, Wv, Wo)
    res = run_bass_kernel_spmd(nc, in_maps, core_ids=list(range(N_CORES)),
                               trace=True)
    return _combine(res.results, bo, x.shape[0]), res


# revision 41
# speedup vs baseline: 1.0397x; 1.0245x over previous
"""Multi-head causal self-attention on 8 Trainium2 NeuronCores.

Problem: B=4, T=2048, D=1024, H=16 heads, Hd=64. fp32.
Sharding: core c handles batch b = c//2 and head-group g = c%2 (8 heads,
512 channels). Each core computes a partial output (its head-group's
contribution to x @ Wo); the host sums head-group pairs and adds bo.

Per-core algorithm (all layouts chosen so no on-chip transposes are
needed after the single x -> x^T transpose):
  x^T  [D=1024, T]   via XBAR DMA transpose (weights are DMA'd first so
                     nothing queues behind the 16 transposes)
  Q^T  [C=512, T]    = matmul(lhsT=Wq chunk, rhs=x^T)   (head h at partitions
  K^T  [C=512, T]      64*(h%2) .. of chunk h//2)
  V'   [T, 8*65]     = matmul(lhsT=x^T chunk, rhs=Wv), per head [V(64) | 1]
  S^T  [k,q]         = matmul(lhsT=K^T block, rhs=Q^T span); the two heads
                       of a pair are row-tiled (partitions 0-63 / 64-127)
                       and run concurrently in the PE array
  E = exp(S^T/8)     on ScalarE, PSUM->SBUF; diagonal blocks get a
                     multiplicative staircase mask on their first 128
                     columns only (the rest of the window is always valid)
  ctx' [65, q]       = matmul(lhsT=V' block, rhs=E)  accumulated over k blocks
                       row 64 = softmax denominator (ones-column trick)
  ctx^T normalized via reciprocal + DRAM-bounce partition broadcast; the
                     ctx' PSUM accumulators are copied to SBUF first so the
                     single PSUM bank per head frees immediately
  out  [T, D]        = matmul(lhsT=ctx^T chunk, rhs=Wo chunk), bf16 DMA out

Phase C (attention) is ScalarE-exp-throughput-bound (~1us/k-block), so
the emission order feeds the Tile scheduler coarse filler blocks it can
slot into C's PE idle time:
  - K/Q/V' groups for (hp0, span s) are emitted just before C(hp0, s), so
    the first exp starts ~25us into the kernel;
  - QK groups for hp 1..3 are emitted between the C(hp) phases and get
    scheduled into earlier C spans' PE gaps;
  - hp3 runs its spans DESCENDING with the output projection for span s
    emitted right after norm(3, s), so out-proj overlaps the remaining
    attention work instead of serializing at the end.
PSUM budget: shared V'/QK/out-proj pool 2 banks + S^T 4 + ctx' 2 = 8.
Causality: only k-blocks with k0 <= q_span_end are computed.
"""

import sys

for _p in ("/opt/trn_rl_repo", "/root/.axon_site/_ro/trn_rl_repo"):
    if _p not in sys.path:
        sys.path.append(_p)

import numpy as np

import concourse.bacc as bacc
import concourse.mybir as mybir
import concourse.tile as tile
from concourse.bass_utils import run_bass_kernel_spmd

FP32 = mybir.dt.float32
BF16 = mybir.dt.bfloat16
P = 128
T = 2048  # sequence length
D = 1024  # model dim
C = 512   # channels per core (8 heads)
H = 8     # heads per core
HD = 64   # head dim
N_CORES = 8
NSPAN = 4          # q spans of 512
SPAN = 512
NKB = 16           # k blocks of 128

_program = None


def _build():
    nc = bacc.Bacc()
    # x arrives pre-transposed AND pre-tiled from the host as
    # [span, 128 d-part, 8 d-chunk, 512 t] so each span's x^T is one
    # fully contiguous 1MB DMA (no XBAR transpose, no strided descriptors).
    x_d = nc.declare_dram_parameter("x", [NSPAN, P, 8, SPAN], BF16,
                                    isOutput=False)
    wq_d = nc.declare_dram_parameter("wq", [D, C], BF16, isOutput=False)
    wk_d = nc.declare_dram_parameter("wk", [D, C], BF16, isOutput=False)
    wv_d = nc.declare_dram_parameter("wv", [D, C], BF16, isOutput=False)
    wo_d = nc.declare_dram_parameter("wo", [C, D], BF16, isOutput=False)
    # mask is [128, 1024] (only cols 0:128 used) so its DMA moves 2KB per
    # partition -- a [128, 128] layout is descriptor-dominated (~13us)
    mask_d = nc.declare_dram_parameter("mask", [P, 1024], BF16, isOutput=False)
    out_d = nc.declare_dram_parameter("out", [T, D], BF16, isOutput=True)

    Exp = mybir.ActivationFunctionType.Exp

    from contextlib import ExitStack

    with tile.TileContext(nc) as tc, ExitStack() as persist:
        const_pool = persist.enter_context(tc.tile_pool(name="const", bufs=1))
        qkt_pool = persist.enter_context(tc.tile_pool(name="qkt", bufs=1))
        vp_pool = persist.enter_context(tc.tile_pool(name="vp", bufs=1))
        persist_w = persist.enter_context(tc.tile_pool(name="pw", bufs=1))
        ctxT_pool = persist.enter_context(tc.tile_pool(name="ctxT", bufs=1))
        xt_pool = persist.enter_context(tc.tile_pool(name="xt", bufs=1))

        # ---- persistent SBUF tiles --------------------------------------
        mask_sb = const_pool.tile([P, 1024], BF16, tag="mask")
        wv_sb = persist_w.tile([P, 8, C], BF16, tag="wv")
        wq_sb = persist_w.tile([P, 8, C], BF16, tag="wq")
        wk_sb = persist_w.tile([P, 8, C], BF16, tag="wk")
        wo_sb = persist_w.tile([P, 4, D], BF16, tag="wo")
        qt = [qkt_pool.tile([P, T], BF16, tag=f"qt{i}", name=f"qt{i}") for i in range(4)]
        kt = [qkt_pool.tile([P, T], BF16, tag=f"kt{i}", name=f"kt{i}") for i in range(4)]
        vp = [vp_pool.tile([P, H * 65], BF16, tag=f"vp{t}", name=f"vp{t}") for t in range(NKB)]
        ctxT = [ctxT_pool.tile([P, T], BF16, tag=f"ct{i}", name=f"ct{i}")
                for i in range(4)]
        # x^T as one tile per q-span: [128 d-part, 8 d-chunk, 512 t].
        # One 1MB DMA per span matches the per-span consumption pattern
        # (DMA completion lags issue by several us and the 8 semaphore
        # lanes serialize on reuse, so fewer/bigger DMAs win at startup).
        xsp = [xt_pool.tile([P, 8, SPAN], BF16, tag=f"xs{s}", name=f"xs{s}")
               for s in range(NSPAN)]
        warm_sb = const_pool.tile([P, SPAN], BF16, tag="warm")

        # ---- startup DMAs: one ring, strict consumption order (startup
        # is HBM-bandwidth-bound at ~8MB total, so what matters is that
        # the first-needed 3MB -- wk, wq, x span 0 -- transfer first).
        nc.sync.dma_start(wk_sb[:], wk_d.rearrange("(o p) c -> p o c", p=P))
        nc.sync.dma_start(wq_sb[:], wq_d.rearrange("(o p) c -> p o c", p=P))
        nc.sync.dma_start(xsp[0][:], x_d[0])
        nc.sync.dma_start(wv_sb[:], wv_d.rearrange("(o p) c -> p o c", p=P))
        nc.sync.dma_start(mask_sb[:], mask_d[:])
        for s in range(1, NSPAN):
            nc.sync.dma_start(xsp[s][:], x_d[s])
        nc.sync.dma_start(wo_sb[:], wo_d.rearrange("(o p) d -> p o d", p=P))

        # ones columns of V' (value 1.0 at element 64 of each head block);
        # warm_sb feeds the PE warm-up burst below.
        nc.gpsimd.memset(warm_sb[:], 1.0)
        for t in range(NKB):
            nc.gpsimd.memset(vp[t][:], 1.0)

        with (
            tc.tile_pool(name="proj", bufs=2, space="PSUM") as proj_pool,
            tc.tile_pool(name="stps", bufs=2, space="PSUM") as st_pool,
            tc.tile_pool(name="csA", bufs=1, space="PSUM") as csA_pool,
            tc.tile_pool(name="csB", bufs=1, space="PSUM") as csB_pool,
            tc.tile_pool(name="epool", bufs=8) as e_pool,
            tc.tile_pool(name="npool", bufs=2) as n_pool,
            tc.tile_pool(name="rdram", bufs=2, space="DRAM") as rdram_pool,
            tc.tile_pool(name="opool", bufs=2) as o_pool,
        ):
            def emit_warmup(n, lo=256, hi=SPAN):
                # dummy matmuls to keep the PE HAM clock gate at 2.4 GHz
                # across stretches where the PE would otherwise idle; the
                # rhs column slice controls what (if anything) they wait on
                w = hi - lo
                ps = proj_pool.tile([P, SPAN], FP32, tag="proj")
                for r in range(n):
                    nc.tensor.matmul(ps[:, 0:w], warm_sb[:, 0:P],
                                     warm_sb[:, lo:hi],
                                     start=(r == 0), stop=(r == n - 1))

            def emit_vprime(t):
                # V' for token block t: [128t, 8*65] with ones col at 64
                sp, tc = t // 4, t % 4
                ps = proj_pool.tile([P, C], FP32, tag="proj")
                for j in range(8):
                    nc.tensor.matmul(
                        ps[:],
                        xsp[sp][:, j, tc * P:(tc + 1) * P],
                        wv_sb[:, j, :],
                        start=(j == 0), stop=(j == 7),
                    )
                dst = vp[t].rearrange("p (h e) -> p h e", e=65)[:, :, 0:64]
                nc.vector.tensor_copy(dst, ps.rearrange("p (h e) -> p h e", e=64))

            def emit_qk_group(dst, wsb, hp, s):
                ps = proj_pool.tile([P, SPAN], FP32, tag="proj")
                for j in range(8):
                    nc.tensor.matmul(
                        ps[:],
                        wsb[:, j, hp * P:(hp + 1) * P],
                        xsp[s][:, j, :],
                        start=(j == 0), stop=(j == 7),
                    )
                nc.vector.tensor_copy(dst[hp][:, s * SPAN:(s + 1) * SPAN], ps[:])

            def emit_attn_span(hp, s, last=False):
                hA, hB = 2 * hp, 2 * hp + 1
                csA = csA_pool.tile([P, SPAN], FP32, tag="csA")
                csB = csB_pool.tile([P, SPAN], FP32, tag="csB")
                nkb = 4 * s + 4
                for kb in range(nkb):
                    ksl = slice(kb * P, (kb + 1) * P)
                    d = max(0, kb - 4 * s)      # diagonal offset 0..3
                    q0 = s * SPAN + 128 * d     # valid q start
                    w = SPAN - 128 * d          # valid width
                    qsl = slice(q0, (s + 1) * SPAN)
                    st = st_pool.tile([P, 1024], FP32, tag="st")
                    st3 = st.rearrange("p (b q) -> p b q", b=2)[:, :, 0:w]
                    # the two heads run concurrently (row-tiled at
                    # partitions 0-63 / 64-127)
                    nc.tensor.matmul(st[:, 0:w], kt[hp][0:64, ksl],
                                     qt[hp][0:64, qsl],
                                     start=True, stop=True)
                    nc.tensor.matmul(st[:, 512:512 + w], kt[hp][64:128, ksl],
                                     qt[hp][64:128, qsl],
                                     start=True, stop=True)
                    e = e_pool.tile([P, 1024], BF16, tag="e")
                    e3 = e.rearrange("p (b q) -> p b q", b=2)[:, :, 0:w]
                    nc.scalar.activation(e3, st3, Exp, scale=0.125)
                    if d > 0 or kb == 4 * s:
                        # staircase only affects the first 128 columns of
                        # the valid window (beyond that q-k >= 128 always)
                        e3m = e.rearrange("p (b q) -> p b q", b=2)[:, :, 0:128]
                        m3 = mask_sb[:, None, 0:128]
                        nc.vector.tensor_mul(
                            e3m, e3m, m3.to_broadcast((P, 2, 128)))
                    co = 128 * d
                    nc.tensor.matmul(csA[0:65, co:SPAN],
                                     vp[kb][:, hA * 65:(hA + 1) * 65],
                                     e[:, 0:w],
                                     start=(kb == 0), stop=(kb == nkb - 1))
                    nc.tensor.matmul(csB[0:65, co:SPAN],
                                     vp[kb][:, hB * 65:(hB + 1) * 65],
                                     e[:, 512:512 + w],
                                     start=(kb == 0), stop=(kb == nkb - 1))
                # Copy the accumulators to SBUF immediately so the csA/csB
                # banks free for the next span; normalize from the copy.
                # rows 0..63 / row 64 (ones-column rowsum).
                # reciprocal_approx_fast is broken at nonzero base
                # partition: broadcast first (DRAM bounce), recip at 0.
                qsl = slice(s * SPAN, (s + 1) * SPAN)
                cs = n_pool.tile([P, 1024], FP32, tag="cs")
                rsAB = n_pool.tile([P, 1024], FP32, tag="rsAB")
                rrAB = n_pool.tile([P, 1024], FP32, tag="rrAB")
                tmpB = n_pool.tile([P, SPAN], BF16, tag="tmpB")
                if last:
                    # final span: nothing reuses csA/csB, so only the
                    # denominator row moves to SBUF and the muls read the
                    # PSUM accumulators directly (shortest tail chain)
                    nc.vector.tensor_copy(cs[64:65, 0:512], csA[64:65, :])
                    nc.vector.tensor_copy(cs[64:65, 512:1024], csB[64:65, :])
                else:
                    nc.vector.tensor_copy(cs[0:65, 0:512], csA[0:65, :])
                    nc.vector.tensor_copy(cs[0:65, 512:1024], csB[0:65, :])
                rd = rdram_pool.tile([1024], FP32, tag="rd")
                nc.sync.dma_start(rd[None, :], cs[64:65, :])
                nc.sync.dma_start(
                    rsAB[0:64, :], rd[None, :].to_broadcast((64, 1024)))
                if last:
                    # re-warm trigger: a tiny cast that lands mid-chain so
                    # the dependent warm-up matmuls bridge the rest of the
                    # normalization latency (see emit_warmup)
                    nc.vector.tensor_copy(warm_sb[0:1, 0:256],
                                          rsAB[0:1, 0:256])
                nc.vector.reciprocal_approx_fast(rrAB[0:64, :], rsAB[0:64, :])
                srcA = csA[0:64, :] if last else cs[0:64, 0:512]
                srcB = csB[0:64, :] if last else cs[0:64, 512:1024]
                nc.vector.tensor_mul(ctxT[hp][0:64, qsl],
                                     srcA, rrAB[0:64, 0:512])
                nc.vector.tensor_mul(tmpB[0:64, :],
                                     srcB, rrAB[0:64, 512:1024])
                nc.sync.dma_start(ctxT[hp][64:128, qsl], tmpB[0:64, :])

            def emit_out_span(s):
                # output projection for the 4 token blocks of span s
                for qb in range(4 * s, 4 * s + 4):
                    ot = o_pool.tile([P, 2, SPAN], BF16, tag="ot")
                    for nh in range(2):
                        ps = proj_pool.tile([P, SPAN], FP32, tag="proj")
                        for hp in range(4):
                            nc.tensor.matmul(
                                ps[:],
                                ctxT[hp][:, qb * P:(qb + 1) * P],
                                wo_sb[:, hp, nh * SPAN:(nh + 1) * SPAN],
                                start=(hp == 0), stop=(hp == 3),
                            )
                        nc.vector.tensor_copy(ot[:, nh, :], ps[:])
                    eng = nc.scalar if s == NSPAN - 1 and qb % 2 == 1 else nc.sync
                    eng.dma_start(
                        out_d[qb * P:(qb + 1) * P, :],
                        ot.rearrange("p a b -> p (a b)"))

            # ---- emission schedule ------------------------------------
            # hp-major (emission order IS per-engine execution order, so
            # keep per-boundary lead-ins small): each head pair's K/Q
            # groups are emitted per span just before that span's
            # attention; hp3 runs its spans DESCENDING with the output
            # projection for span s right after norm(3, s) so out-proj
            # overlaps the remaining attention instead of the tail.
            emit_warmup(28)
            for s in range(NSPAN):
                emit_qk_group(kt, wk_sb, 0, s)
                emit_qk_group(qt, wq_sb, 0, s)
                for t in range(4 * s, 4 * s + 4):
                    emit_vprime(t)
                emit_attn_span(0, s)
            for hp in (1, 2):
                for s in range(NSPAN):
                    emit_qk_group(kt, wk_sb, hp, s)
                    emit_qk_group(qt, wq_sb, hp, s)
                    emit_attn_span(hp, s)
            for s in range(NSPAN):
                emit_qk_group(kt, wk_sb, 3, s)
                emit_qk_group(qt, wq_sb, 3, s)
            for s in reversed(range(NSPAN)):
                emit_attn_span(3, s, last=(s == 0))
                if s == 0:
                    # bridge the final norm-chain latency so the PE stays
                    # warm for the last output-projection burst: part one
                    # free-runs after the last ctx, part two waits on the
                    # mid-chain trigger written by the final norm
                    emit_warmup(12, 256, SPAN)      # free-running
                    emit_warmup(16, 0, 256)        # gated on the trigger
                emit_out_span(s)

    nc.compile()
    return nc


def _get_program():
    global _program
    if _program is None:
        _program = _build()
    return _program


def _make_mask():
    import ml_dtypes
    j = np.arange(1024)[None, :]
    k = np.arange(P)[:, None]
    return np.where((j >= k) | (j >= P), 1.0, 0.0).astype(ml_dtypes.bfloat16)


def _make_in_maps(x, Wq, Wk, Wv, Wo):
    import ml_dtypes
    bf16 = ml_dtypes.bfloat16
    mask = _make_mask()
    in_maps = []
    xp = {}
    for b in range(x.shape[0]):
        # [T, D] -> x^T tiled as [span, 128 d-part, 8 d-chunk, 512 t]
        xT = np.asarray(x[b], np.float32).astype(bf16).T  # [D, T]
        xp[b] = np.ascontiguousarray(
            xT.reshape(8, P, NSPAN, SPAN).transpose(2, 1, 0, 3))
    for c in range(N_CORES):
        b, g = c // 2, c % 2
        cols = slice(g * C, (g + 1) * C)
        in_maps.append({
            "x": xp[b],
            "wq": np.ascontiguousarray(np.asarray(Wq[:, cols], np.float32).astype(bf16)),
            "wk": np.ascontiguousarray(np.asarray(Wk[:, cols], np.float32).astype(bf16)),
            "wv": np.ascontiguousarray(np.asarray(Wv[:, cols], np.float32).astype(bf16)),
            "wo": np.ascontiguousarray(np.asarray(Wo[cols, :], np.float32).astype(bf16)),
            "mask": mask,
        })
    return in_maps


def _combine(results, bo, B):
    out = np.empty((B, T, D), dtype=np.float32)
    bo = np.asarray(bo, dtype=np.float32)
    for b in range(B):
        out[b] = (results[2 * b]["out"].astype(np.float32)
                  + results[2 * b + 1]["out"].astype(np.float32) + bo)
    return out


def kernel(x, Wq, Wk, Wv, Wo, bo):
    x = np.asarray(x)
    nc = _get_program()
    in_maps = _make_in_maps(x, Wq, Wk, Wv, Wo)
    res = run_bass_kernel_spmd(nc, in_maps, core_ids=list(range(N_CORES)))
    return _combine(res.results, bo, x.shape[0])


def kernel_traced(x, Wq, Wk, Wv, Wo, bo):
    """Like kernel() but also returns the BassKernelResults (with
    exec_time_ns when NTFF tracing is available)."""
    x = np.asarray(x)
    nc = _get_program()
    in_maps = _make_in_maps(x, Wq, Wk, Wv, Wo)
    res = run_bass_kernel_spmd(nc, in_maps, core_ids=list(range(N_CORES)),
                               trace=True)
    return _combine(res.results, bo, x.shape[0]), res


# revision 42
# speedup vs baseline: 1.0419x; 1.0021x over previous
"""Multi-head causal self-attention on 8 Trainium2 NeuronCores.

Problem: B=4, T=2048, D=1024, H=16 heads, Hd=64. fp32.
Sharding: core c handles batch b = c//2 and head-group g = c%2 (8 heads,
512 channels). Each core computes a partial output (its head-group's
contribution to x @ Wo); the host sums head-group pairs and adds bo.

Per-core algorithm (all layouts chosen so no on-chip transposes are
needed after the single x -> x^T transpose):
  x^T  [D=1024, T]   via XBAR DMA transpose (weights are DMA'd first so
                     nothing queues behind the 16 transposes)
  Q^T  [C=512, T]    = matmul(lhsT=Wq chunk, rhs=x^T)   (head h at partitions
  K^T  [C=512, T]      64*(h%2) .. of chunk h//2)
  V'   [T, 8*65]     = matmul(lhsT=x^T chunk, rhs=Wv), per head [V(64) | 1]
  S^T  [k,q]         = matmul(lhsT=K^T block, rhs=Q^T span); the two heads
                       of a pair are row-tiled (partitions 0-63 / 64-127)
                       and run concurrently in the PE array
  E = exp(S^T/8)     on ScalarE, PSUM->SBUF; diagonal blocks get a
                     multiplicative staircase mask on their first 128
                     columns only (the rest of the window is always valid)
  ctx' [65, q]       = matmul(lhsT=V' block, rhs=E)  accumulated over k blocks
                       row 64 = softmax denominator (ones-column trick)
  ctx^T normalized via reciprocal + DRAM-bounce partition broadcast; the
                     ctx' PSUM accumulators are copied to SBUF first so the
                     single PSUM bank per head frees immediately
  out  [T, D]        = matmul(lhsT=ctx^T chunk, rhs=Wo chunk), bf16 DMA out

Phase C (attention) is ScalarE-exp-throughput-bound (~1us/k-block), so
the emission order feeds the Tile scheduler coarse filler blocks it can
slot into C's PE idle time:
  - K/Q/V' groups for (hp0, span s) are emitted just before C(hp0, s), so
    the first exp starts ~25us into the kernel;
  - QK groups for hp 1..3 are emitted between the C(hp) phases and get
    scheduled into earlier C spans' PE gaps;
  - hp3 runs its spans DESCENDING with the output projection for span s
    emitted right after norm(3, s), so out-proj overlaps the remaining
    attention work instead of serializing at the end.
PSUM budget: shared V'/QK/out-proj pool 2 banks + S^T 4 + ctx' 2 = 8.
Causality: only k-blocks with k0 <= q_span_end are computed.
"""

import sys

for _p in ("/opt/trn_rl_repo", "/root/.axon_site/_ro/trn_rl_repo"):
    if _p not in sys.path:
        sys.path.append(_p)

import numpy as np

import concourse.bacc as bacc
import concourse.mybir as mybir
import concourse.tile as tile
from concourse.bass_utils import run_bass_kernel_spmd

FP32 = mybir.dt.float32
BF16 = mybir.dt.bfloat16
P = 128
T = 2048  # sequence length
D = 1024  # model dim
C = 512   # channels per core (8 heads)
H = 8     # heads per core
HD = 64   # head dim
N_CORES = 8
NSPAN = 4          # q spans of 512
SPAN = 512
NKB = 16           # k blocks of 128

_program = None


def _build():
    nc = bacc.Bacc()
    # x arrives pre-transposed AND pre-tiled from the host as
    # [span, 128 d-part, 8 d-chunk, 512 t] so each span's x^T is one
    # fully contiguous 1MB DMA (no XBAR transpose, no strided descriptors).
    x_d = nc.declare_dram_parameter("x", [NSPAN, P, 8, SPAN], BF16,
                                    isOutput=False)
    wq_d = nc.declare_dram_parameter("wq", [D, C], BF16, isOutput=False)
    wk_d = nc.declare_dram_parameter("wk", [D, C], BF16, isOutput=False)
    wv_d = nc.declare_dram_parameter("wv", [D, C], BF16, isOutput=False)
    wo_d = nc.declare_dram_parameter("wo", [C, D], BF16, isOutput=False)
    mask_d = nc.declare_dram_parameter("mask", [P, P], BF16, isOutput=False)
    out_d = nc.declare_dram_parameter("out", [T, D], BF16, isOutput=True)

    Exp = mybir.ActivationFunctionType.Exp

    from contextlib import ExitStack

    with tile.TileContext(nc) as tc, ExitStack() as persist:
        const_pool = persist.enter_context(tc.tile_pool(name="const", bufs=1))
        qkt_pool = persist.enter_context(tc.tile_pool(name="qkt", bufs=1))
        vp_pool = persist.enter_context(tc.tile_pool(name="vp", bufs=1))
        persist_w = persist.enter_context(tc.tile_pool(name="pw", bufs=1))
        ctxT_pool = persist.enter_context(tc.tile_pool(name="ctxT", bufs=1))
        xt_pool = persist.enter_context(tc.tile_pool(name="xt", bufs=1))

        # ---- persistent SBUF tiles --------------------------------------
        mask_sb = const_pool.tile([P, P], BF16, tag="mask")
        wv_sb = persist_w.tile([P, 8, C], BF16, tag="wv")
        wq_sb = persist_w.tile([P, 8, C], BF16, tag="wq")
        wk_sb = persist_w.tile([P, 8, C], BF16, tag="wk")
        wo_sb = persist_w.tile([P, 4, D], BF16, tag="wo")
        qt = [qkt_pool.tile([P, T], BF16, tag=f"qt{i}", name=f"qt{i}") for i in range(4)]
        kt = [qkt_pool.tile([P, T], BF16, tag=f"kt{i}", name=f"kt{i}") for i in range(4)]
        vp = [vp_pool.tile([P, H * 65], BF16, tag=f"vp{t}", name=f"vp{t}") for t in range(NKB)]
        ctxT = [ctxT_pool.tile([P, T], BF16, tag=f"ct{i}", name=f"ct{i}")
                for i in range(4)]
        # x^T as one tile per q-span: [128 d-part, 8 d-chunk, 512 t].
        # One 1MB DMA per span matches the per-span consumption pattern
        # (DMA completion lags issue by several us and the 8 semaphore
        # lanes serialize on reuse, so fewer/bigger DMAs win at startup).
        xsp = [xt_pool.tile([P, 8, SPAN], BF16, tag=f"xs{s}", name=f"xs{s}")
               for s in range(NSPAN)]
        warm_sb = const_pool.tile([P, SPAN], BF16, tag="warm")

        # ---- startup DMAs: one ring, strict consumption order (startup
        # is HBM-bandwidth-bound at ~8MB total, so what matters is that
        # the first-needed 3MB -- wk, wq, x span 0 -- transfer first).
        nc.sync.dma_start(wk_sb[:], wk_d.rearrange("(o p) c -> p o c", p=P))
        nc.sync.dma_start(wq_sb[:], wq_d.rearrange("(o p) c -> p o c", p=P))
        nc.sync.dma_start(xsp[0][:], x_d[0])
        nc.sync.dma_start(wv_sb[:], wv_d.rearrange("(o p) c -> p o c", p=P))
        nc.sync.dma_start(mask_sb[:], mask_d[:])
        for s in range(1, NSPAN):
            nc.sync.dma_start(xsp[s][:], x_d[s])
        nc.sync.dma_start(wo_sb[:], wo_d.rearrange("(o p) d -> p o d", p=P))

        # ones columns of V' (value 1.0 at element 64 of each head block);
        # warm_sb feeds the PE warm-up burst below.
        nc.gpsimd.memset(warm_sb[:], 1.0)
        for t in range(NKB):
            nc.gpsimd.memset(vp[t][:], 1.0)

        with (
            tc.tile_pool(name="proj", bufs=2, space="PSUM") as proj_pool,
            tc.tile_pool(name="stps", bufs=2, space="PSUM") as st_pool,
            tc.tile_pool(name="csA", bufs=1, space="PSUM") as csA_pool,
            tc.tile_pool(name="csB", bufs=1, space="PSUM") as csB_pool,
            tc.tile_pool(name="epool", bufs=6) as e_pool,
            tc.tile_pool(name="npool", bufs=2) as n_pool,
            tc.tile_pool(name="rdram", bufs=2, space="DRAM") as rdram_pool,
            tc.tile_pool(name="opool", bufs=2) as o_pool,
        ):
            def emit_warmup(n, lo=256, hi=SPAN):
                # dummy matmuls to keep the PE HAM clock gate at 2.4 GHz
                # across stretches where the PE would otherwise idle; the
                # rhs column slice controls what (if anything) they wait on
                w = hi - lo
                ps = proj_pool.tile([P, SPAN], FP32, tag="proj")
                for r in range(n):
                    nc.tensor.matmul(ps[:, 0:w], warm_sb[:, 0:P],
                                     warm_sb[:, lo:hi],
                                     start=(r == 0), stop=(r == n - 1))

            def emit_vprime(t):
                # V' for token block t: [128t, 8*65] with ones col at 64
                sp, tc = t // 4, t % 4
                ps = proj_pool.tile([P, C], FP32, tag="proj")
                for j in range(8):
                    nc.tensor.matmul(
                        ps[:],
                        xsp[sp][:, j, tc * P:(tc + 1) * P],
                        wv_sb[:, j, :],
                        start=(j == 0), stop=(j == 7),
                    )
                dst = vp[t].rearrange("p (h e) -> p h e", e=65)[:, :, 0:64]
                nc.vector.tensor_copy(dst, ps.rearrange("p (h e) -> p h e", e=64))

            def emit_qk_group(dst, wsb, hp, s):
                ps = proj_pool.tile([P, SPAN], FP32, tag="proj")
                for j in range(8):
                    nc.tensor.matmul(
                        ps[:],
                        wsb[:, j, hp * P:(hp + 1) * P],
                        xsp[s][:, j, :],
                        start=(j == 0), stop=(j == 7),
                    )
                nc.vector.tensor_copy(dst[hp][:, s * SPAN:(s + 1) * SPAN], ps[:])

            def emit_attn_span(hp, s, last=False):
                hA, hB = 2 * hp, 2 * hp + 1
                csA = csA_pool.tile([P, SPAN], FP32, tag="csA")
                csB = csB_pool.tile([P, SPAN], FP32, tag="csB")
                nkb = 4 * s + 4
                for kb in range(nkb):
                    ksl = slice(kb * P, (kb + 1) * P)
                    d = max(0, kb - 4 * s)      # diagonal offset 0..3
                    q0 = s * SPAN + 128 * d     # valid q start
                    w = SPAN - 128 * d          # valid width
                    qsl = slice(q0, (s + 1) * SPAN)
                    st = st_pool.tile([P, 1024], FP32, tag="st")
                    st3 = st.rearrange("p (b q) -> p b q", b=2)[:, :, 0:w]
                    # the two heads run concurrently (row-tiled at
                    # partitions 0-63 / 64-127)
                    nc.tensor.matmul(st[:, 0:w], kt[hp][0:64, ksl],
                                     qt[hp][0:64, qsl],
                                     start=True, stop=True)
                    nc.tensor.matmul(st[:, 512:512 + w], kt[hp][64:128, ksl],
                                     qt[hp][64:128, qsl],
                                     start=True, stop=True)
                    e = e_pool.tile([P, 1024], BF16, tag="e")
                    e3 = e.rearrange("p (b q) -> p b q", b=2)[:, :, 0:w]
                    nc.scalar.activation(e3, st3, Exp, scale=0.125)
                    if d > 0 or kb == 4 * s:
                        # staircase only affects the first 128 columns of
                        # the valid window (beyond that q-k >= 128 always)
                        e3m = e.rearrange("p (b q) -> p b q", b=2)[:, :, 0:128]
                        m3 = mask_sb[:, None, :]
                        nc.vector.tensor_mul(
                            e3m, e3m, m3.to_broadcast((P, 2, 128)))
                    co = 128 * d
                    nc.tensor.matmul(csA[0:65, co:SPAN],
                                     vp[kb][:, hA * 65:(hA + 1) * 65],
                                     e[:, 0:w],
                                     start=(kb == 0), stop=(kb == nkb - 1))
                    nc.tensor.matmul(csB[0:65, co:SPAN],
                                     vp[kb][:, hB * 65:(hB + 1) * 65],
                                     e[:, 512:512 + w],
                                     start=(kb == 0), stop=(kb == nkb - 1))
                # Copy the accumulators to SBUF immediately so the csA/csB
                # banks free for the next span; normalize from the copy.
                # rows 0..63 / row 64 (ones-column rowsum).
                # reciprocal_approx_fast is broken at nonzero base
                # partition: broadcast first (DRAM bounce), recip at 0.
                qsl = slice(s * SPAN, (s + 1) * SPAN)
                cs = n_pool.tile([P, 1024], FP32, tag="cs")
                rsAB = n_pool.tile([P, 1024], FP32, tag="rsAB")
                rrAB = n_pool.tile([P, 1024], FP32, tag="rrAB")
                tmpB = n_pool.tile([P, SPAN], BF16, tag="tmpB")
                nc.vector.tensor_copy(cs[0:65, 0:512], csA[0:65, :])
                nc.vector.tensor_copy(cs[0:65, 512:1024], csB[0:65, :])
                rd = rdram_pool.tile([1024], FP32, tag="rd")
                nc.sync.dma_start(rd[None, :], cs[64:65, :])
                nc.sync.dma_start(
                    rsAB[0:64, :], rd[None, :].to_broadcast((64, 1024)))
                nc.vector.reciprocal_approx_fast(rrAB[0:64, :], rsAB[0:64, :])
                nc.vector.tensor_mul(ctxT[hp][0:64, qsl],
                                     cs[0:64, 0:512], rrAB[0:64, 0:512])
                nc.vector.tensor_mul(tmpB[0:64, :],
                                     cs[0:64, 512:1024], rrAB[0:64, 512:1024])
                nc.sync.dma_start(ctxT[hp][64:128, qsl], tmpB[0:64, :])

            def emit_out_span(s):
                # output projection for the 4 token blocks of span s
                for qb in range(4 * s, 4 * s + 4):
                    ot = o_pool.tile([P, 2, SPAN], BF16, tag="ot")
                    for nh in range(2):
                        ps = proj_pool.tile([P, SPAN], FP32, tag="proj")
                        for hp in range(4):
                            nc.tensor.matmul(
                                ps[:],
                                ctxT[hp][:, qb * P:(qb + 1) * P],
                                wo_sb[:, hp, nh * SPAN:(nh + 1) * SPAN],
                                start=(hp == 0), stop=(hp == 3),
                            )
                        nc.vector.tensor_copy(ot[:, nh, :], ps[:])
                    eng = nc.scalar if s == NSPAN - 1 and qb % 2 == 1 else nc.sync
                    eng.dma_start(
                        out_d[qb * P:(qb + 1) * P, :],
                        ot.rearrange("p a b -> p (a b)"))

            # ---- emission schedule ------------------------------------
            # hp-major (emission order IS per-engine execution order, so
            # keep per-boundary lead-ins small): each head pair's K/Q
            # groups are emitted per span just before that span's
            # attention; hp3 runs its spans DESCENDING with the output
            # projection for span s right after norm(3, s) so out-proj
            # overlaps the remaining attention instead of the tail.
            emit_warmup(28, 0, SPAN)
            for s in range(NSPAN):
                emit_qk_group(kt, wk_sb, 0, s)
                emit_qk_group(qt, wq_sb, 0, s)
                for t in range(4 * s, 4 * s + 4):
                    emit_vprime(t)
                emit_attn_span(0, s)
            for hp in (1, 2):
                for s in range(NSPAN):
                    emit_qk_group(kt, wk_sb, hp, s)
                    emit_qk_group(qt, wq_sb, hp, s)
                    emit_attn_span(hp, s)
            for s in range(NSPAN):
                emit_qk_group(kt, wk_sb, 3, s)
                emit_qk_group(qt, wq_sb, 3, s)
            for s in reversed(range(NSPAN)):
                emit_attn_span(3, s, last=(s == 0))
                emit_out_span(s)

    nc.compile()
    return nc


def _get_program():
    global _program
    if _program is None:
        _program = _build()
    return _program


def _make_mask():
    import ml_dtypes
    j = np.arange(P)[None, :]
    k = np.arange(P)[:, None]
    return np.where(j >= k, 1.0, 0.0).astype(ml_dtypes.bfloat16)


def _make_in_maps(x, Wq, Wk, Wv, Wo):
    import ml_dtypes
    bf16 = ml_dtypes.bfloat16
    mask = _make_mask()
    in_maps = []
    xp = {}
    for b in range(x.shape[0]):
        # [T, D] -> x^T tiled as [span, 128 d-part, 8 d-chunk, 512 t]
        xT = np.asarray(x[b], np.float32).astype(bf16).T  # [D, T]
        xp[b] = np.ascontiguousarray(
            xT.reshape(8, P, NSPAN, SPAN).transpose(2, 1, 0, 3))
    for c in range(N_CORES):
        b, g = c // 2, c % 2
        cols = slice(g * C, (g + 1) * C)
        in_maps.append({
            "x": xp[b],
            "wq": np.ascontiguousarray(np.asarray(Wq[:, cols], np.float32).astype(bf16)),
            "wk": np.ascontiguousarray(np.asarray(Wk[:, cols], np.float32).astype(bf16)),
            "wv": np.ascontiguousarray(np.asarray(Wv[:, cols], np.float32).astype(bf16)),
            "wo": np.ascontiguousarray(np.asarray(Wo[cols, :], np.float32).astype(bf16)),
            "mask": mask,
        })
    return in_maps


def _combine(results, bo, B):
    out = np.empty((B, T, D), dtype=np.float32)
    bo = np.asarray(bo, dtype=np.float32)
    for b in range(B):
        out[b] = (results[2 * b]["out"].astype(np.float32)
                  + results[2 * b + 1]["out"].astype(np.float32) + bo)
    return out


def kernel(x, Wq, Wk, Wv, Wo, bo):
    x = np.asarray(x)
    nc = _get_program()
    in_maps = _make_in_maps(x, Wq, Wk, Wv, Wo)
    res = run_bass_kernel_spmd(nc, in_maps, core_ids=list(range(N_CORES)))
    return _combine(res.results, bo, x.shape[0])


def kernel_traced(x, Wq, Wk, Wv, Wo, bo):
    """Like kernel() but also returns the BassKernelResults (with
    exec_time_ns when NTFF tracing is available)."""
    x = np.asarray(x)
    nc = _get_program()
    in_maps = _make_in_maps(x, Wq, Wk, Wv, Wo)
    res = run_bass_kernel_spmd(nc, in_maps, core_ids=list(range(N_CORES)),
                               trace=True)
    return _combine(res.results, bo, x.shape[0]), res
